# revision 25
# baseline (speedup 1.0000x reference)
"""DeltaNet-style fast-weight kernel for Trainium2 (8 NeuronCores, data-parallel over batch).

Math (per batch element b):
  h = embed[seq]; x = h + MLP(h); h = LN(x)                      [L=512 tokens, H=64]
  keys k_t = h[t], t=0..510 ; kn_t = k_t/||k_t||
  M_t = M_{t-1}(I - kn_t kn_t^T) + k_t kn_t^T ; y = M_510 @ h[511]
  out = (y @ rp_w + rp_b) @ out_w + out_b

Key reductions:
1. y = sum_t c_t k_t with a backward vector scan
     z_{510} = q;  c_t = kn_t . z_t;  z_{t-1} = z_t - c_t kn_t
   (algebraically identical to the reference M-scan; the kernel tracks
   zneg = -z so both scan ops are fused multiply-adds).
2. Every per-token quantity is a function of the token id alone
   (vocab = 64): the whole embed+MLP+LN+normalize pipeline is computed
   once for the 64 vocab rows, and per-token values are gathered with
   one-hot matmuls  ohT_chunk @ [kntab | I]  ->  [kn_t | oh_t] rows.
3. The y-accumulation is folded into the scan: state W = [zneg | ybins],
   op2 updates both halves in one 128-wide op (ybins[s_t] += c_t), and
   y = ybins @ htab afterwards.
4. (y @ rp_w + rp_b) @ out_w + out_b collapses into one matmul against
   a precomputed [htab @ rp_w @ out_w ; bias] matrix.

Scan state and gathered streams are fp16 (c accumulated in fp32): DVE
runs 2-elem/cycle on 2-byte dtypes, and fp16 keeps the end-to-end
relative error ~2.5e-3 (verified offline vs the fp64 reference).
Tile builds run in descending l order so the PE/ACT/DMA gather pipeline
overlaps the DVE scan of the previous tile.
"""

import os
import sys

import numpy as np

for _p in ("/opt/trn_rl_repo", "/root/.axon_site/_ro/trn_rl_repo"):
    if os.path.isdir(_p) and _p not in sys.path:
        sys.path.insert(0, _p)

import concourse.bass as bass
import concourse.tile as tile
from concourse import bacc, mybir
from concourse.bass_utils import run_bass_kernel_spmd
from concourse.masks import make_identity

F32 = mybir.dt.float32
F16 = mybir.dt.float16
I32 = mybir.dt.int32
AF = mybir.ActivationFunctionType
OP = mybir.AluOpType

B, L, H, V = 256, 512, 64, 64
NCORES = 8
BS = B // NCORES          # 32 batches per core
LT = 16                   # l-values per tile
NT = L // LT              # 32 tiles
LN_EPS = 1e-5


def _ap(dram_ap, offset, dims):
    """Raw access pattern on a DRAM tensor."""
    return bass.AP(tensor=dram_ap.tensor, offset=offset, ap=dims)


def build_program():
    nc = bacc.Bacc(None, target_bir_lowering=False)

    seq_p = nc.declare_dram_parameter("seq", [BS, L], I32, isOutput=False)
    embed_p = nc.declare_dram_parameter("embed", [V, H], F32, isOutput=False)
    w1_p = nc.declare_dram_parameter("w1", [H, 2 * H], F32, isOutput=False)
    b1_p = nc.declare_dram_parameter("b1", [2 * H, 1], F32, isOutput=False)
    w2_p = nc.declare_dram_parameter("w2", [2 * H, H], F32, isOutput=False)
    b2_p = nc.declare_dram_parameter("b2", [H, 1], F32, isOutput=False)
    ln_g_p = nc.declare_dram_parameter("ln_g", [1, H], F32, isOutput=False)
    ln_b_p = nc.declare_dram_parameter("ln_b", [1, H], F32, isOutput=False)
    rp_w_p = nc.declare_dram_parameter("rp_w", [H, H], F32, isOutput=False)
    rp_b_p = nc.declare_dram_parameter("rp_b", [H, 1], F32, isOutput=False)
    out_w_p = nc.declare_dram_parameter("out_w", [H, V], F32, isOutput=False)
    out_b_p = nc.declare_dram_parameter("out_b", [1, V], F32, isOutput=False)
    out_p = nc.declare_dram_parameter("out", [BS, V], F32, isOutput=True)

    # seq as fp16 in DRAM so the v-partition broadcast DMA reads 2-byte values
    seq16_d = nc.dram_tensor("seq16_scratch", [BS, L], F16)

    from contextlib import ExitStack

    with tile.TileContext(nc) as tc, ExitStack() as ctx:
        consts = ctx.enter_context(tc.tile_pool(name="consts", bufs=1))
        big = ctx.enter_context(tc.tile_pool(name="big", bufs=1))
        work = ctx.enter_context(tc.tile_pool(name="work", bufs=3))
        ps_g = ctx.enter_context(tc.tile_pool(name="ps_g", bufs=4, space="PSUM"))
        ps_m = ctx.enter_context(tc.tile_pool(name="ps_m", bufs=1, space="PSUM"))

        # ---------------- constants / params ----------------
        ident = consts.tile([H, H], F32)
        make_identity(nc, ident)

        eps_sb = consts.tile([V, 1], F32)
        nc.vector.memset(eps_sb, LN_EPS)

        viota_i = consts.tile([V, 1], I32)
        nc.gpsimd.iota(viota_i, pattern=[[1, 1]], base=0, channel_multiplier=1)
        viota = consts.tile([V, 1], F32)
        nc.vector.tensor_copy(viota, viota_i)

        embed_sb = consts.tile([V, H], F32)
        w1_sb = consts.tile([H, 2 * H], F32)
        b1_sb = consts.tile([2 * H, 1], F32)
        w2_sb = consts.tile([2 * H, H], F32)
        b2_sb = consts.tile([H, 1], F32)
        rp_w_sb = consts.tile([H, H], F32)
        rp_b_sb = consts.tile([H, 1], F32)
        out_w_sb = consts.tile([H, V], F32)
        out_b_sb = consts.tile([1, V], F32)
        for sb, p in (
            (embed_sb, embed_p), (w1_sb, w1_p), (b1_sb, b1_p), (w2_sb, w2_p),
            (b2_sb, b2_p), (rp_w_sb, rp_w_p), (rp_b_sb, rp_b_p),
            (out_w_sb, out_w_p), (out_b_sb, out_b_p),
        ):
            nc.sync.dma_start(out=sb, in_=p[:, :])

        g_bc = consts.tile([V, H], F32)
        bta_bc = consts.tile([V, H], F32)
        nc.sync.dma_start(
            out=g_bc,
            in_=_ap(ln_g_p[0, :], 0, [[0, V], *ln_g_p[0, :].ap]),
        )
        nc.sync.dma_start(
            out=bta_bc,
            in_=_ap(ln_b_p[0, :], 0, [[0, V], *ln_b_p[0, :].ap]),
        )

        # seq -> fp16, transpose to [L, BS] via PE, stash in DRAM
        seq_i = consts.tile([BS, L], I32)
        nc.sync.dma_start(out=seq_i, in_=seq_p[:, :])
        seq16_sb = consts.tile([BS, L], F16)
        nc.vector.tensor_copy(seq16_sb, seq_i)

        # PE transpose-mode matmuls depend only on the gpsimd-built identity;
        # this throwaway advances PE's observed Pool clock so later matmuls
        # need at most one semaphore wait each.
        dummy_ps = ps_m.tile([BS, BS], F32, tag="psm_dummy")
        nc.tensor.matmul(dummy_ps, lhsT=ident[0:BS, 0:BS], rhs=ident[0:BS, 0:BS], start=True, stop=True)

        ident16 = consts.tile([H, H], F16)
        nc.vector.tensor_copy(ident16, ident)

        nc.sync.dma_start(out=seq16_d[:, :], in_=seq16_sb)

        # Priority broadcast of tile 31's slice (small, strided) so the scan
        # can start before the big flat broadcast below finishes streaming.
        seqb31 = big.tile([V, BS, LT], F16)
        nc.sync.dma_start(
            out=seqb31,
            in_=_ap(seq16_d[0, :], LT * (NT - 1), [[0, V], [L, BS], [1, LT]]),
        )
        # seqball[v, b, l] = seq[b, l]: one flat DMA, contiguous 32KB per
        # partition — the most descriptor-efficient form (l-split variants
        # with 256-512B runs measured slower end-to-end).
        seqball = big.tile([V, BS, L], F16)
        nc.sync.dma_start(
            out=seqball,
            in_=_ap(seq16_d[0, :], 0, [[0, V], [1, BS * L]]),
        )

        # ---------------- 64-entry tables ----------------
        pse = ps_m.tile([H, V], F32, tag="psm")
        nc.tensor.matmul(pse, lhsT=embed_sb, rhs=ident, start=True, stop=True)
        embedT = consts.tile([H, V], F32)
        nc.vector.tensor_copy(embedT, pse)

        psa = ps_m.tile([2 * H, V], F32, tag="psm2")
        nc.tensor.matmul(psa, lhsT=w1_sb, rhs=embedT, start=True, stop=True)
        rT = consts.tile([2 * H, V], F32)
        nc.scalar.activation(rT, psa, AF.Relu, bias=b1_sb[:, 0:1])

        psx = ps_m.tile([H, V], F32, tag="psm")
        nc.tensor.matmul(psx, lhsT=w2_sb, rhs=rT, start=True, stop=False)
        nc.tensor.matmul(psx, lhsT=ident, rhs=embedT, start=False, stop=True)
        xT = consts.tile([H, V], F32)
        nc.scalar.activation(xT, psx, AF.Identity, bias=b2_sb[:, 0:1])

        psx2 = ps_m.tile([V, H], F32, tag="psm2")
        nc.tensor.matmul(psx2, lhsT=xT, rhs=ident, start=True, stop=True)
        x_sb = consts.tile([V, H], F32)
        nc.vector.tensor_copy(x_sb, psx2)

        st = consts.tile([V, 6], F32)
        mv = consts.tile([V, 2], F32)
        nc.vector.bn_stats(st, x_sb)
        nc.vector.bn_aggr(mv, st)
        sstd = consts.tile([V, 1], F32)
        rstd = consts.tile([V, 1], F32)
        nc.scalar.activation(sstd, mv[:, 1:2], AF.Sqrt, bias=eps_sb[:, 0:1])
        nc.vector.reciprocal(rstd, sstd)

        htab = consts.tile([V, H], F32)
        nc.vector.tensor_scalar(
            out=htab, in0=x_sb, scalar1=mv[:, 0:1], scalar2=rstd[:, 0:1],
            op0=OP.subtract, op1=OP.mult,
        )
        nc.vector.tensor_mul(htab, htab, g_bc)
        nc.vector.tensor_add(htab, htab, bta_bc)

        sq = consts.tile([V, H], F32)
        ss = consts.tile([V, 1], F32)
        nc.vector.scalar_tensor_tensor(
            out=sq, in0=htab, scalar=1.0, in1=htab,
            op0=OP.mult, op1=OP.mult, accum_out=ss,
        )
        sn = consts.tile([V, 1], F32)
        rn = consts.tile([V, 1], F32)
        nc.scalar.activation(sn, ss, AF.Sqrt)
        nc.vector.tensor_scalar(sn, sn, 1e-12, None, op0=OP.max)
        nc.vector.reciprocal(rn, sn)

        # Atab = [kntab | I] fp16; htabneg fp16 for the z0 gather
        Atab16 = consts.tile([V, 2 * H], F16)
        nc.vector.tensor_scalar(
            out=Atab16[:, 0:H], in0=htab, scalar1=rn[:, 0:1], scalar2=None,
            op0=OP.mult,
        )
        nc.vector.tensor_copy(Atab16[:, H:2 * H], ident16)
        htabneg16 = consts.tile([V, H], F16)
        nc.vector.tensor_scalar(htabneg16, htab, -1.0, None, op0=OP.mult)

        # HWb = [htab @ rp_w @ out_w ; rp_b @ out_w + out_b]  [65, V]
        psht = ps_m.tile([H, V], F32, tag="psm")
        nc.tensor.matmul(psht, lhsT=htab, rhs=ident, start=True, stop=True)
        htabT = consts.tile([H, V], F32)
        nc.vector.tensor_copy(htabT, psht)

        psrw = ps_m.tile([H, H], F32, tag="psm2")
        nc.tensor.matmul(psrw, lhsT=rp_w_sb, rhs=ident, start=True, stop=True)
        rp_wT = consts.tile([H, H], F32)
        nc.vector.tensor_copy(rp_wT, psrw)

        psw2 = ps_m.tile([H, V], F32, tag="psm")
        nc.tensor.matmul(psw2, lhsT=rp_wT, rhs=out_w_sb, start=True, stop=True)
        W2_sb = consts.tile([H, V], F32)
        nc.vector.tensor_copy(W2_sb, psw2)

        HWb = consts.tile([H + 1, V], F32)
        pshw = ps_m.tile([V, V], F32, tag="psm2")
        nc.tensor.matmul(pshw, lhsT=htabT, rhs=W2_sb, start=True, stop=True)
        nc.scalar.activation(HWb[0:H, :], pshw, AF.Copy)

        psbias = ps_m.tile([1, V], F32, tag="psm")
        nc.tensor.matmul(psbias, lhsT=rp_b_sb, rhs=out_w_sb, start=True, stop=True)
        bias_sb = consts.tile([1, V], F32)
        nc.vector.tensor_copy(bias_sb, psbias)
        nc.vector.tensor_add(HWb[H:H + 1, :], bias_sb, out_b_sb)

        # ---------------- scan state ----------------
        Wst = big.tile([BS, 2 * H], F16)          # [zneg | ybins]
        nc.vector.memset(Wst, 0.0)
        c_sb = big.tile([BS, L], F32)
        u16 = big.tile([BS, H], F16)
        akn = [
            big.tile([BS, LT, 2 * H], F16, name=f"akn{i}", tag=f"akn{i}")
            for i in range(NT)
        ]

        # ---------------- tile builds + interleaved scan ----------------
        for i in range(NT - 1, -1, -1):
            ohT = work.tile([V, BS, LT], F16)
            src = seqb31 if i == NT - 1 else seqball[:, :, LT * i:LT * (i + 1)]
            nc.vector.tensor_scalar(
                out=ohT, in0=src, scalar1=viota[:, 0:1],
                scalar2=None, op0=OP.is_equal,
            )
            for j in range(4):
                # lhsT = 8 batches x 16 l_offs (contiguous 128 columns);
                # psg partition p = 16*b_rel + l_off
                psg = ps_g.tile([4 * BS, 2 * H], F32, tag="psg")
                nc.tensor.matmul(
                    psg, lhsT=ohT[:, 8 * j:8 * (j + 1), :], rhs=Atab16,
                    start=True, stop=True,
                )
                stg = work.tile([4 * BS, 2 * H], F16)
                nc.scalar.activation(stg, psg, AF.Copy)
                nc.sync.dma_start(out=akn[i][8 * j:8 * (j + 1), :, :], in_=stg)

            if i == NT - 1:
                # zneg0 = -htab[seq[:, 511]]  (l_off 15 of tile 31)
                psz = ps_m.tile([BS, H], F32, tag="psm")
                nc.tensor.matmul(
                    psz, lhsT=ohT[:, :, 15:16], rhs=htabneg16,
                    start=True, stop=True,
                )
                nc.scalar.activation(Wst[:, 0:H], psz, AF.Copy)

            l_hi = min(LT * i + LT - 1, L - 2)
            for l in range(l_hi, LT * i - 1, -1):
                row = akn[i][:, l - LT * i, :]
                nc.vector.scalar_tensor_tensor(
                    out=u16, in0=row[:, 0:H], scalar=-1.0, in1=Wst[:, 0:H],
                    op0=OP.mult, op1=OP.mult, accum_out=c_sb[:, l:l + 1],
                )
                nc.vector.scalar_tensor_tensor(
                    out=Wst, in0=row, scalar=c_sb[:, l:l + 1], in1=Wst,
                    op0=OP.mult, op1=OP.add,
                )

        # ---------------- tail: out = [ybins | 1] @ HWb ----------------
        psyt = ps_m.tile([H, BS], F32, tag="psm")
        nc.tensor.matmul(
            psyt, lhsT=Wst[:, H:2 * H], rhs=ident16[0:BS, 0:BS],
            start=True, stop=True,
        )
        yb_sb = big.tile([H + 1, BS], F32)
        nc.scalar.activation(yb_sb[0:H, :], psyt, AF.Copy)
        nc.vector.memset(yb_sb[H:H + 1, :], 1.0)

        pso = ps_m.tile([BS, V], F32, tag="psm2")
        nc.tensor.matmul(pso, lhsT=yb_sb, rhs=HWb, start=True, stop=True)
        o_sb = big.tile([BS, V], F32)
        nc.scalar.activation(o_sb, pso, AF.Copy)
        nc.sync.dma_start(out=out_p[:, :], in_=o_sb)

    nc.finalize()
    return nc


_CACHE = {}


def _run(inputs, trace=False, **kw):
    seq = np.asarray(inputs["seq"]).astype(np.int32)
    embed = np.asarray(inputs["embed"], np.float32)
    w1 = np.asarray(inputs["w1"], np.float32)
    b1 = np.asarray(inputs["b1"], np.float32).reshape(2 * H, 1)
    w2 = np.asarray(inputs["w2"], np.float32)
    b2 = np.asarray(inputs["b2"], np.float32).reshape(H, 1)
    ln_g = np.asarray(inputs["ln_g"], np.float32).reshape(1, H)
    ln_b = np.asarray(inputs["ln_b"], np.float32).reshape(1, H)
    rp_w = np.asarray(inputs["rp_w"], np.float32)
    rp_b = np.asarray(inputs["rp_b"], np.float32).reshape(H, 1)
    out_w = np.asarray(inputs["out_w"], np.float32)
    out_b = np.asarray(inputs["out_b"], np.float32).reshape(1, V)

    if "nc" not in _CACHE:
        _CACHE["nc"] = build_program()
    nc = _CACHE["nc"]

    in_maps = []
    for c in range(NCORES):
        in_maps.append({
            "seq": seq[BS * c:BS * (c + 1)],
            "embed": embed, "w1": w1, "b1": b1, "w2": w2, "b2": b2,
            "ln_g": ln_g, "ln_b": ln_b,
            "rp_w": rp_w, "rp_b": rp_b, "out_w": out_w, "out_b": out_b,
        })
    br = run_bass_kernel_spmd(nc, in_maps, list(range(NCORES)), trace=trace, **kw)
    out = np.concatenate([r["out"] for r in br.results], axis=0)
    return out, br


def kernel(**inputs) -> np.ndarray:
    return _run(inputs)[0]


# revision 27
# speedup vs baseline: 1.1764x; 1.1764x over previous
"""DeltaNet-style fast-weight kernel for Trainium2 (8 NeuronCores, data-parallel over batch).

Math (per batch element b):
  h = embed[seq]; x = h + MLP(h); h = LN(x)                      [L=512 tokens, H=64]
  keys k_t = h[t], t=0..510 ; kn_t = k_t/||k_t||
  M_t = M_{t-1}(I - kn_t kn_t^T) + k_t kn_t^T ; y = M_510 @ h[511]
  out = (y @ rp_w + rp_b) @ out_w + out_b

Key reductions:
1. y = sum_t c_t k_t with a backward vector scan
     z_{510} = q;  c_t = kn_t . z_t;  z_{t-1} = z_t - c_t kn_t
   (algebraically identical to the reference M-scan; the kernel tracks
   zneg = -z so both scan ops are fused multiply-adds).
2. Every per-token quantity is a function of the token id alone
   (vocab = 64): the whole embed+MLP+LN+normalize pipeline is computed
   once for the 64 vocab rows, and per-token values are gathered with
   one-hot matmuls  ohT_chunk @ [kntab | I]  ->  [kn_t | oh_t] rows.
3. The y-accumulation is folded into the scan: state W = [zneg | ybins],
   op2 updates both halves in one 128-wide op (ybins[s_t] += c_t), and
   y = ybins @ htab afterwards.
4. (y @ rp_w + rp_b) @ out_w + out_b collapses into one matmul against
   a precomputed [htab @ rp_w @ out_w ; bias] matrix.

Scan state and gathered streams are fp16 (c accumulated in fp32): DVE
runs 2-elem/cycle on 2-byte dtypes, and fp16 keeps the end-to-end
relative error ~2.5e-3 (verified offline vs the fp64 reference).
Tile builds run in descending l order so the PE/ACT/DMA gather pipeline
overlaps the DVE scan of the previous tile.
"""

import os
import sys

import numpy as np

for _p in ("/opt/trn_rl_repo", "/root/.axon_site/_ro/trn_rl_repo"):
    if os.path.isdir(_p) and _p not in sys.path:
        sys.path.insert(0, _p)

import concourse.bass as bass
import concourse.tile as tile
from concourse import bacc, mybir
from concourse.bass_utils import run_bass_kernel_spmd
from concourse.masks import make_identity

F32 = mybir.dt.float32
F16 = mybir.dt.float16
I32 = mybir.dt.int32
AF = mybir.ActivationFunctionType
OP = mybir.AluOpType

B, L, H, V = 256, 512, 64, 64
NCORES = 8
BS = B // NCORES          # 32 batches per core
LT = 16                   # l-values per tile
NT = L // LT              # 32 tiles
LN_EPS = 1e-5


def _ap(dram_ap, offset, dims):
    """Raw access pattern on a DRAM tensor."""
    return bass.AP(tensor=dram_ap.tensor, offset=offset, ap=dims)


def build_program():
    nc = bacc.Bacc(None, target_bir_lowering=False)

    seq_p = nc.declare_dram_parameter("seq", [BS, L], I32, isOutput=False)
    embed_p = nc.declare_dram_parameter("embed", [V, H], F32, isOutput=False)
    w1_p = nc.declare_dram_parameter("w1", [H, 2 * H], F32, isOutput=False)
    b1_p = nc.declare_dram_parameter("b1", [2 * H, 1], F32, isOutput=False)
    w2_p = nc.declare_dram_parameter("w2", [2 * H, H], F32, isOutput=False)
    b2_p = nc.declare_dram_parameter("b2", [H, 1], F32, isOutput=False)
    ln_g_p = nc.declare_dram_parameter("ln_g", [1, H], F32, isOutput=False)
    ln_b_p = nc.declare_dram_parameter("ln_b", [1, H], F32, isOutput=False)
    rp_w_p = nc.declare_dram_parameter("rp_w", [H, H], F32, isOutput=False)
    rp_b_p = nc.declare_dram_parameter("rp_b", [H, 1], F32, isOutput=False)
    out_w_p = nc.declare_dram_parameter("out_w", [H, V], F32, isOutput=False)
    out_b_p = nc.declare_dram_parameter("out_b", [1, V], F32, isOutput=False)
    out_p = nc.declare_dram_parameter("out", [BS, V], F32, isOutput=True)

    # seq as fp16 in DRAM so the v-partition broadcast DMA reads 2-byte values
    seq16_d = nc.dram_tensor("seq16_scratch", [BS, L], F16)

    from contextlib import ExitStack

    with tile.TileContext(nc) as tc, ExitStack() as ctx:
        consts = ctx.enter_context(tc.tile_pool(name="consts", bufs=1))
        big = ctx.enter_context(tc.tile_pool(name="big", bufs=1))
        work = ctx.enter_context(tc.tile_pool(name="work", bufs=3))
        ps_g = ctx.enter_context(tc.tile_pool(name="ps_g", bufs=4, space="PSUM"))
        ps_m = ctx.enter_context(tc.tile_pool(name="ps_m", bufs=1, space="PSUM"))

        # ---------------- constants / params ----------------
        ident = consts.tile([H, H], F32)
        make_identity(nc, ident)

        eps_sb = consts.tile([V, 1], F32)
        nc.vector.memset(eps_sb, LN_EPS)

        viota_i = consts.tile([V, 1], I32)
        nc.gpsimd.iota(viota_i, pattern=[[1, 1]], base=0, channel_multiplier=1)
        viota = consts.tile([V, 1], F32)
        nc.vector.tensor_copy(viota, viota_i)

        embed_sb = consts.tile([V, H], F32)
        w1_sb = consts.tile([H, 2 * H], F32)
        b1_sb = consts.tile([2 * H, 1], F32)
        w2_sb = consts.tile([2 * H, H], F32)
        b2_sb = consts.tile([H, 1], F32)
        rp_w_sb = consts.tile([H, H], F32)
        rp_b_sb = consts.tile([H, 1], F32)
        out_w_sb = consts.tile([H, V], F32)
        out_b_sb = consts.tile([1, V], F32)
        for sb, p in (
            (embed_sb, embed_p), (w1_sb, w1_p), (b1_sb, b1_p), (w2_sb, w2_p),
            (b2_sb, b2_p), (rp_w_sb, rp_w_p), (rp_b_sb, rp_b_p),
            (out_w_sb, out_w_p), (out_b_sb, out_b_p),
        ):
            nc.sync.dma_start(out=sb, in_=p[:, :])

        g_bc = consts.tile([V, H], F32)
        bta_bc = consts.tile([V, H], F32)
        nc.sync.dma_start(
            out=g_bc,
            in_=_ap(ln_g_p[0, :], 0, [[0, V], *ln_g_p[0, :].ap]),
        )
        nc.sync.dma_start(
            out=bta_bc,
            in_=_ap(ln_b_p[0, :], 0, [[0, V], *ln_b_p[0, :].ap]),
        )

        # seq -> fp16, transpose to [L, BS] via PE, stash in DRAM
        seq_i = consts.tile([BS, L], I32)
        nc.sync.dma_start(out=seq_i, in_=seq_p[:, :])
        seq16_sb = consts.tile([BS, L], F16)
        nc.vector.tensor_copy(seq16_sb, seq_i)

        # PE transpose-mode matmuls depend only on the gpsimd-built identity;
        # this throwaway advances PE's observed Pool clock so later matmuls
        # need at most one semaphore wait each.
        dummy_ps = ps_m.tile([BS, BS], F32, tag="psm_dummy")
        nc.tensor.matmul(dummy_ps, lhsT=ident[0:BS, 0:BS], rhs=ident[0:BS, 0:BS], start=True, stop=True)

        ident16 = consts.tile([H, H], F16)
        nc.vector.tensor_copy(ident16, ident)

        nc.sync.dma_start(out=seq16_d[:, :], in_=seq16_sb)

        # Priority broadcast of tiles 31/30/29 (small, strided) so the scan
        # runs ~3 tiles before the big flat broadcast below finishes streaming.
        NPRI = 3
        seqbpri = big.tile([V, NPRI, BS, LT], F16)
        for k in range(NPRI):
            nc.sync.dma_start(
                out=seqbpri[:, k, :, :],
                in_=_ap(
                    seq16_d[0, :], LT * (NT - 1 - k),
                    [[0, V], [L, BS], [1, LT]],
                ),
            )
        # seqball[v, b, l] = seq[b, l]: one flat DMA, contiguous 32KB per
        # partition — the most descriptor-efficient form (l-split variants
        # with 256-512B runs measured slower end-to-end).
        seqball = big.tile([V, BS, L], F16)
        nc.sync.dma_start(
            out=seqball,
            in_=_ap(seq16_d[0, :], 0, [[0, V], [1, BS * L]]),
        )

        # ---------------- 64-entry tables ----------------
        pse = ps_m.tile([H, V], F32, tag="psm")
        nc.tensor.matmul(pse, lhsT=embed_sb, rhs=ident, start=True, stop=True)
        embedT = consts.tile([H, V], F32)
        nc.vector.tensor_copy(embedT, pse)

        psa = ps_m.tile([2 * H, V], F32, tag="psm2")
        nc.tensor.matmul(psa, lhsT=w1_sb, rhs=embedT, start=True, stop=True)
        rT = consts.tile([2 * H, V], F32)
        nc.scalar.activation(rT, psa, AF.Relu, bias=b1_sb[:, 0:1])

        psx = ps_m.tile([H, V], F32, tag="psm")
        nc.tensor.matmul(psx, lhsT=w2_sb, rhs=rT, start=True, stop=False)
        nc.tensor.matmul(psx, lhsT=ident, rhs=embedT, start=False, stop=True)
        xT = consts.tile([H, V], F32)
        nc.scalar.activation(xT, psx, AF.Identity, bias=b2_sb[:, 0:1])

        psx2 = ps_m.tile([V, H], F32, tag="psm2")
        nc.tensor.matmul(psx2, lhsT=xT, rhs=ident, start=True, stop=True)
        x_sb = consts.tile([V, H], F32)
        nc.vector.tensor_copy(x_sb, psx2)

        st = consts.tile([V, 6], F32)
        mv = consts.tile([V, 2], F32)
        nc.vector.bn_stats(st, x_sb)
        nc.vector.bn_aggr(mv, st)
        sstd = consts.tile([V, 1], F32)
        rstd = consts.tile([V, 1], F32)
        nc.scalar.activation(sstd, mv[:, 1:2], AF.Sqrt, bias=eps_sb[:, 0:1])
        nc.vector.reciprocal(rstd, sstd)

        htab = consts.tile([V, H], F32)
        nc.vector.tensor_scalar(
            out=htab, in0=x_sb, scalar1=mv[:, 0:1], scalar2=rstd[:, 0:1],
            op0=OP.subtract, op1=OP.mult,
        )
        nc.vector.tensor_mul(htab, htab, g_bc)
        nc.vector.tensor_add(htab, htab, bta_bc)

        sq = consts.tile([V, H], F32)
        ss = consts.tile([V, 1], F32)
        nc.vector.scalar_tensor_tensor(
            out=sq, in0=htab, scalar=1.0, in1=htab,
            op0=OP.mult, op1=OP.mult, accum_out=ss,
        )
        sn = consts.tile([V, 1], F32)
        rn = consts.tile([V, 1], F32)
        nc.scalar.activation(sn, ss, AF.Sqrt)
        nc.vector.tensor_scalar(sn, sn, 1e-12, None, op0=OP.max)
        nc.vector.reciprocal(rn, sn)

        # Atab = [kntab | I] fp16; htabneg fp16 for the z0 gather
        Atab16 = consts.tile([V, 2 * H], F16)
        nc.vector.tensor_scalar(
            out=Atab16[:, 0:H], in0=htab, scalar1=rn[:, 0:1], scalar2=None,
            op0=OP.mult,
        )
        nc.vector.tensor_copy(Atab16[:, H:2 * H], ident16)
        htabneg16 = consts.tile([V, H], F16)
        nc.vector.tensor_scalar(htabneg16, htab, -1.0, None, op0=OP.mult)

        # HWb = [htab @ rp_w @ out_w ; rp_b @ out_w + out_b]  [65, V]
        psht = ps_m.tile([H, V], F32, tag="psm")
        nc.tensor.matmul(psht, lhsT=htab, rhs=ident, start=True, stop=True)
        htabT = consts.tile([H, V], F32)
        nc.vector.tensor_copy(htabT, psht)

        psrw = ps_m.tile([H, H], F32, tag="psm2")
        nc.tensor.matmul(psrw, lhsT=rp_w_sb, rhs=ident, start=True, stop=True)
        rp_wT = consts.tile([H, H], F32)
        nc.vector.tensor_copy(rp_wT, psrw)

        psw2 = ps_m.tile([H, V], F32, tag="psm")
        nc.tensor.matmul(psw2, lhsT=rp_wT, rhs=out_w_sb, start=True, stop=True)
        W2_sb = consts.tile([H, V], F32)
        nc.vector.tensor_copy(W2_sb, psw2)

        HWb = consts.tile([H + 1, V], F32)
        pshw = ps_m.tile([V, V], F32, tag="psm2")
        nc.tensor.matmul(pshw, lhsT=htabT, rhs=W2_sb, start=True, stop=True)
        nc.scalar.activation(HWb[0:H, :], pshw, AF.Copy)

        psbias = ps_m.tile([1, V], F32, tag="psm")
        nc.tensor.matmul(psbias, lhsT=rp_b_sb, rhs=out_w_sb, start=True, stop=True)
        bias_sb = consts.tile([1, V], F32)
        nc.vector.tensor_copy(bias_sb, psbias)
        nc.vector.tensor_add(HWb[H:H + 1, :], bias_sb, out_b_sb)

        # ---------------- scan state ----------------
        Wst = big.tile([BS, 2 * H], F16)          # [zneg | ybins]
        nc.vector.memset(Wst, 0.0)
        c_sb = big.tile([BS, L], F32)
        u16 = big.tile([BS, H], F16)
        akn = [
            big.tile([BS, LT, 2 * H], F16, name=f"akn{i}", tag=f"akn{i}")
            for i in range(NT)
        ]

        # ---------------- tile builds + interleaved scan ----------------
        for i in range(NT - 1, -1, -1):
            ohT = work.tile([V, BS, LT], F16)
            if i >= NT - NPRI:
                src = seqbpri[:, NT - 1 - i, :, :]
            else:
                src = seqball[:, :, LT * i:LT * (i + 1)]
            nc.vector.tensor_scalar(
                out=ohT, in0=src, scalar1=viota[:, 0:1],
                scalar2=None, op0=OP.is_equal,
            )
            for j in range(4):
                # lhsT = 8 batches x 16 l_offs (contiguous 128 columns);
                # psg partition p = 16*b_rel + l_off
                psg = ps_g.tile([4 * BS, 2 * H], F32, tag="psg")
                nc.tensor.matmul(
                    psg, lhsT=ohT[:, 8 * j:8 * (j + 1), :], rhs=Atab16,
                    start=True, stop=True,
                )
                stg = work.tile([4 * BS, 2 * H], F16)
                nc.scalar.activation(stg, psg, AF.Copy)
                nc.sync.dma_start(out=akn[i][8 * j:8 * (j + 1), :, :], in_=stg)

            if i == NT - 1:
                # zneg0 = -htab[seq[:, 511]]  (l_off 15 of tile 31)
                psz = ps_m.tile([BS, H], F32, tag="psm")
                nc.tensor.matmul(
                    psz, lhsT=ohT[:, :, 15:16], rhs=htabneg16,
                    start=True, stop=True,
                )
                nc.scalar.activation(Wst[:, 0:H], psz, AF.Copy)

            l_hi = min(LT * i + LT - 1, L - 2)
            for l in range(l_hi, LT * i - 1, -1):
                row = akn[i][:, l - LT * i, :]
                nc.vector.scalar_tensor_tensor(
                    out=u16, in0=row[:, 0:H], scalar=-1.0, in1=Wst[:, 0:H],
                    op0=OP.mult, op1=OP.mult, accum_out=c_sb[:, l:l + 1],
                )
                nc.vector.scalar_tensor_tensor(
                    out=Wst, in0=row, scalar=c_sb[:, l:l + 1], in1=Wst,
                    op0=OP.mult, op1=OP.add,
                )

        # ---------------- tail: out = [ybins | 1] @ HWb ----------------
        psyt = ps_m.tile([H, BS], F32, tag="psm")
        nc.tensor.matmul(
            psyt, lhsT=Wst[:, H:2 * H], rhs=ident16[0:BS, 0:BS],
            start=True, stop=True,
        )
        yb_sb = big.tile([H + 1, BS], F32)
        nc.scalar.activation(yb_sb[0:H, :], psyt, AF.Copy)
        nc.vector.memset(yb_sb[H:H + 1, :], 1.0)

        pso = ps_m.tile([BS, V], F32, tag="psm2")
        nc.tensor.matmul(pso, lhsT=yb_sb, rhs=HWb, start=True, stop=True)
        o_sb = big.tile([BS, V], F32)
        nc.scalar.activation(o_sb, pso, AF.Copy)
        nc.sync.dma_start(out=out_p[:, :], in_=o_sb)

    nc.finalize()
    return nc


_CACHE = {}


def _run(inputs, trace=False, **kw):
    seq = np.asarray(inputs["seq"]).astype(np.int32)
    embed = np.asarray(inputs["embed"], np.float32)
    w1 = np.asarray(inputs["w1"], np.float32)
    b1 = np.asarray(inputs["b1"], np.float32).reshape(2 * H, 1)
    w2 = np.asarray(inputs["w2"], np.float32)
    b2 = np.asarray(inputs["b2"], np.float32).reshape(H, 1)
    ln_g = np.asarray(inputs["ln_g"], np.float32).reshape(1, H)
    ln_b = np.asarray(inputs["ln_b"], np.float32).reshape(1, H)
    rp_w = np.asarray(inputs["rp_w"], np.float32)
    rp_b = np.asarray(inputs["rp_b"], np.float32).reshape(H, 1)
    out_w = np.asarray(inputs["out_w"], np.float32)
    out_b = np.asarray(inputs["out_b"], np.float32).reshape(1, V)

    if "nc" not in _CACHE:
        _CACHE["nc"] = build_program()
    nc = _CACHE["nc"]

    in_maps = []
    for c in range(NCORES):
        in_maps.append({
            "seq": seq[BS * c:BS * (c + 1)],
            "embed": embed, "w1": w1, "b1": b1, "w2": w2, "b2": b2,
            "ln_g": ln_g, "ln_b": ln_b,
            "rp_w": rp_w, "rp_b": rp_b, "out_w": out_w, "out_b": out_b,
        })
    br = run_bass_kernel_spmd(nc, in_maps, list(range(NCORES)), trace=trace, **kw)
    out = np.concatenate([r["out"] for r in br.results], axis=0)
    return out, br


def kernel(**inputs) -> np.ndarray:
    return _run(inputs)[0]
